# revision 4
# baseline (speedup 1.0000x reference)
"""Multi-head attention layer (QKV proj + RoPE + SDPA + o_proj) on 8 TRN2 cores.

Sharding: DP2 x TP4. Core c handles batch c//4 and heads 4*(c%4)..4*(c%4)+4.
Each core computes its 4 heads' attention and a partial o_proj output
[L, D]; the host sums the 4 partials per batch (row-parallel o_proj).

All matmul operands are bf16 (same 1 cycle/row PE rate as fp32r on TRN2,
half the DMA/SBUF footprint); PSUM accumulation is fp32 throughout.

Structure (single PE stream, minimal gaps):
  phase 1: for each 512-token block n, stream htk once and run q, k, v
           projections back-to-back out of the same SBUF tiles. RoPE is
           PE-free: the rotate-half permutation is done with two ACT
           partition-offset copies (sign folded into sin host-side), the
           cos/sin multiplies run on DVE in bf16 (2x mode), and the final
           add runs on GpSimd. Rope work for block n's flush is spread
           into the following projection stream (one unit per kg
           boundary) so no engine queue piles up.
  phase 2: attention, 2-kv-tile supersteps with a 2-step software
           pipeline: scores for tiles (t, t+1) are issued before av(t-2),
           av(t-1), so the exp (ACT) latency hides behind PE work. Scores
           land in a [128,1024] fp32 PSUM tile (two single-shot 512-wide
           matmuls -> one 1024-wide exp; wq carries the 1/sqrt(Hd)
           scale). Softmax skips max-subtraction (scores ~N(0,1)).
           Denominator: probs accumulate into 3 bf16 partials on DVE;
           GpSimd pre-reduces them to one; the PE only does 2 single-shot
           ones-matmuls per (head, half), written into ps_out[0:1] after
           its copy-out (so no PSUM banks are reserved for the
           denominator). Reciprocal on DVE, broadcast across partitions
           with gpsimd.partition_broadcast (no DRAM bounce), in-place DVE
           normalize.
  phase 3: o_proj lives in the same PSUM pool scope as attention, using
           the 2 banks freed by the denominator change: 4-matmul
           accumulation chains alternate between the two banks of one
           [128,1024] tile. A few chain-groups are interleaved into the
           second half of phase 2 (the PE has ~100ns/step slack while ACT
           streams exps); the rest run right after the last score matmul,
           overlapping the attention tail drain. bf16 output DMA (summed
           to fp32 on the host); the last token tile DMAs per-d-block to
           shorten the drain.

Accumulation-chain rule learned the hard way: `start=True` clears the
has_written bits for the WHOLE PSUM bank, so two interleaved multi-step
accumulation chains must never share a bank (single-shot matmuls may).
"""

import numpy as np

import sys
import types

# Defensive: concourse.bass_utils imports antenv.axon_hooks when tracing is
# requested; provide a null shim if the module is absent in this image so a
# stray BASS_TRACE env var cannot crash the kernel.
try:
    import antenv.axon_hooks  # noqa: F401
except ImportError:
    _m = types.ModuleType("antenv.axon_hooks")
    _m.set_axon_ntff_profile_hook = lambda h: None
    _m.get_axon_ntff_profile_hook = lambda: None
    sys.modules["antenv.axon_hooks"] = _m

import ml_dtypes

import concourse.bass as bass
import concourse.mybir as mybir
import concourse.tile as tile
from concourse import bacc
from concourse.bass_utils import run_bass_kernel_spmd

# problem constants (hardcoded per spec)
B, L, D = 2, 2048, 2048
H, Hd = 16, 128
NC = 8
TPH = 4            # heads per core
QKV = TPH * Hd     # 512 per-core projection width
KT = D // 128      # 16 contraction tiles
NT = L // 512      # 4 token groups of 512
MT = L // 128      # 16 token chunks of 128

f32 = mybir.dt.float32
bf16 = mybir.dt.bfloat16

AF = mybir.ActivationFunctionType
SCALE = 1.0 / float(np.sqrt(Hd))

_CACHE: dict = {}


def _build():
    nc = bacc.Bacc("TRN2", target_bir_lowering=False, debug=False)

    # inputs are pre-tiled on the host so every DMA line is contiguous per
    # partition (4-16KB instead of 1KB)
    hTt = nc.dram_tensor("hTt", [NT, 128, KT, 512], bf16, kind="ExternalInput").ap()
    wqT = nc.dram_tensor("wqT", [128, KT, QKV], bf16, kind="ExternalInput").ap()
    wkT = nc.dram_tensor("wkT", [128, KT, QKV], bf16, kind="ExternalInput").ap()
    wvT = nc.dram_tensor("wvT", [128, KT, QKV], bf16, kind="ExternalInput").ap()
    woT = nc.dram_tensor("woT", [128, TPH, D], bf16, kind="ExternalInput").ap()
    cosT = nc.dram_tensor("cosT", [Hd, L], bf16, kind="ExternalInput").ap()
    sinTs = nc.dram_tensor("sinTs", [Hd, L], bf16, kind="ExternalInput").ap()
    out = nc.dram_tensor("out", [L, D], bf16, kind="ExternalOutput").ap()

    out_re = out.rearrange("(mm p) (nb d) -> p mm nb d", p=128, d=512)

    with tile.TileContext(nc) as tc:
        with tc.tile_pool(name="persist", bufs=1) as persist:
            # ---- persistent tensors -----------------------------------
            ones_b = persist.tile([128, 1], bf16, name="ones_b")
            nc.vector.memset(ones_b, 1.0)
            warm = persist.tile([128, 512], bf16, name="warm")
            nc.vector.memset(warm, 0.0)
            qT = [persist.tile([Hd, L], bf16, name=f"qT{h}") for h in range(TPH)]
            kT = [persist.tile([Hd, L], bf16, name=f"kT{h}") for h in range(TPH)]
            v_big = persist.tile([128, MT, QKV], bf16, name="v_big")
            outT = [persist.tile([Hd, L], bf16, name=f"outT{h}") for h in range(TPH)]
            cos_sb = persist.tile([Hd, L], bf16, name="cos_sb")
            sin_sb = persist.tile([Hd, L], bf16, name="sin_sb")

            # ==== phase 1: fused q/k/v projections + RoPE ====================
            with (
                tc.tile_pool(name="wqkv", bufs=1) as wqkv,
                tc.tile_pool(name="stream", bufs=2) as stream,
                tc.tile_pool(name="tmp", bufs=2) as tmp,
                tc.tile_pool(name="pp", bufs=1, space="PSUM") as pp,
            ):
                # first htk block ahead of everything else on the sync queue;
                # leading chunks are fine-grained so the first matmul can
                # start as soon as ktile 0 and wq ktile 0 land
                htk0 = stream.tile([128, KT, 512], bf16, name="htk")
                for a, b in ((0, 1), (1, 2), (2, 4), (4, 8), (8, 16)):
                    nc.sync.dma_start(
                        out=htk0[:, a:b, :],
                        in_=hTt[0, :, a:b, :],
                    )
                # weight prefetch, tag-major: q is consumed first (+4us),
                # then k (+17us), then v (+31us)
                w_sbs = {}
                for tag in ("q", "k", "v"):
                    w_sbs[tag] = wqkv.tile([128, KT, QKV], bf16, name=f"w_{tag}")
                w_res = {"q": wqT, "k": wkT, "v": wvT}
                for tag in ("q", "k", "v"):
                    w_re = w_res[tag]
                    for a, b in (
                        (0, 1), (1, 2), (2, 4), (4, 6), (6, 8),
                        (8, 10), (10, 12), (12, 16),
                    ):
                        nc.gpsimd.dma_start(
                            out=w_sbs[tag][:, a:b, :], in_=w_re[:, a:b, :]
                        )
                # rope constants: needed at the first flush (~14us in)
                nc.sync.dma_start(out=cos_sb, in_=cosT)
                nc.sync.dma_start(out=sin_sb, in_=sinTs)

                # PE warm-up: junk matmuls during the initial DMA wait so the
                # clock ramp (free-running ~3.4us activity window) completes
                # before real data lands
                ps_warm = pp.tile([128, 512], f32, name="pp0", bufs=2)
                for _ in range(5):
                    nc.tensor.matmul(
                        ps_warm, warm[:, 0:128], warm, start=True, stop=True
                    )

                # deferred rope work: list of closures, emitted one per kg
                # boundary of the *following* projection stream so the ACT
                # queue (which also drains the PSUM flush copies) never
                # backs up.
                pending_rope = []

                def emit_one_rope():
                    if pending_rope:
                        pending_rope.pop(0)()

                def make_rope(dst, m, n, raw):
                    csl = slice(n * 512, (n + 1) * 512)

                    def do():
                        # rotate-half permutation: two partition-offset
                        # copies on ACT (sign already folded into sinTs)
                        rr = tmp.tile([128, 512], bf16, name="rr")
                        nc.scalar.copy(rr[0:64, :], raw[64:128, :])
                        nc.scalar.copy(rr[64:128, :], raw[0:64, :])
                        t1 = tmp.tile([128, 512], bf16, name="t1")
                        nc.vector.tensor_mul(t1, raw, cos_sb[:, csl])
                        t2 = tmp.tile([128, 512], bf16, name="t2")
                        nc.vector.tensor_mul(t2, rr, sin_sb[:, csl])
                        nc.gpsimd.tensor_add(dst[m][:, csl], t1, t2)

                    return do

                for n in range(NT):
                    if n == 0:
                        htk = htk0
                    else:
                        htk = stream.tile([128, KT, 512], bf16, name="htk")
                        for kg in range(2):
                            nc.sync.dma_start(
                                out=htk[:, kg * 8 : (kg + 1) * 8, :],
                                in_=hTt[n, :, kg * 8 : (kg + 1) * 8, :],
                            )
                    for tag, dst in (("q", qT), ("k", kT), ("v", None)):
                        w_sb = w_sbs[tag]
                        ps_x = [
                            pp.tile(
                                [128, 512], f32, name=f"pp{m}",
                                bufs=2 if m < 2 else 1,
                            )
                            for m in range(TPH)
                        ]
                        if tag == "v":
                            # v: stationary = token columns of htk, moving = wv
                            # rows; out [token128, qkv512] per token sub-tile.
                            for kg in range(4):
                                for mc in range(4):
                                    for i in range(4):
                                        kk = kg * 4 + i
                                        nc.tensor.matmul(
                                            ps_x[mc],
                                            htk[:, kk, mc * 128 : (mc + 1) * 128],
                                            w_sb[:, kk, :],
                                            start=(kk == 0),
                                            stop=(kk == KT - 1),
                                        )
                                emit_one_rope()
                            for mc in range(4):
                                if mc < 2:
                                    nc.scalar.copy(v_big[:, n * 4 + mc, :], ps_x[mc])
                                else:
                                    nc.vector.tensor_copy(
                                        v_big[:, n * 4 + mc, :], ps_x[mc]
                                    )
                        else:
                            # q/k: stationary = weight m-tile, moving = htk.
                            # m-major within each kg so the first matmul of
                            # m2/m3 (single-buffered banks) comes late enough
                            # for the previous flush to have freed them.
                            for kg in range(4):
                                for m in range(TPH):
                                    for i in range(4):
                                        kk = kg * 4 + i
                                        nc.tensor.matmul(
                                            ps_x[m],
                                            w_sb[:, kk, m * 128 : (m + 1) * 128],
                                            htk[:, kk, :],
                                            start=(kk == 0),
                                            stop=(kk == KT - 1),
                                        )
                                emit_one_rope()
                            # flush: raw copies split ACT/DVE, rope deferred
                            raws = []
                            for m in range(TPH):
                                raw = tmp.tile([128, 512], bf16, name="raw", bufs=4)
                                if m < 2:
                                    nc.scalar.copy(raw, ps_x[m])
                                else:
                                    nc.vector.tensor_copy(raw, ps_x[m])
                                raws.append(raw)
                            for m in range(TPH):
                                pending_rope.append(make_rope(dst, m, n, raws[m]))
                while pending_rope:
                    emit_one_rope()

            # ==== phase 2+3: attention with fused o_proj =====================
            with (
                tc.tile_pool(name="wo", bufs=1) as wop,
                tc.tile_pool(name="att", bufs=2) as att,
            ):
                wo_sb = wop.tile([128, TPH, D], bf16, name="wo_sb")
                for hh in range(TPH):
                    nc.gpsimd.dma_start(out=wo_sb[:, hh, :], in_=woT[:, hh, :])

                HW = 1024  # tq half-width

                seq = [
                    (half, h, tk)
                    for half in range(2)
                    for h in range(TPH)
                    for tk in range(MT)
                ]
                n_seq = len(seq)
                state = {}  # (half,h) -> ps_out
                probs_by_idx = {}
                partials = {}
                # partial index by tk: p0 = tk0-5, p1 = tk6-10, p2 = tk11-15
                P_OF_TK = [0] * 6 + [1] * 5 + [2] * 5

                PIPE = 2  # av trails scores by 2 steps so exp() is done
                with (
                    tc.tile_pool(name="pss", bufs=2, space="PSUM") as pss,
                    tc.tile_pool(name="pso", bufs=1, space="PSUM") as pso,
                    tc.tile_pool(name="pof", bufs=1, space="PSUM") as pof,
                ):
                    def front(t):
                        half, h, tk = seq[t]
                        # full-width scores tile (2 banks); each 512-wide
                        # matmul is single-shot (start&stop) into its own
                        # bank. One 1024-wide exp serves both. wq is
                        # pre-scaled by 1/sqrt(Hd) on the host.
                        sc_ps = pss.tile([128, HW], f32, name="sc")
                        for j in range(2):
                            tq0 = half * HW + j * 512
                            nc.tensor.matmul(
                                sc_ps[:, j * 512 : (j + 1) * 512],
                                kT[h][:, tk * 128 : (tk + 1) * 128],
                                qT[h][:, tq0 : tq0 + 512],
                                start=True,
                                stop=True,
                            )
                        probs = att.tile([128, HW], bf16, name="probs", bufs=5)
                        probs_by_idx[t] = probs
                        nc.scalar.activation(probs, sc_ps, AF.Exp)

                    def back_av(t):
                        half, h, tk = seq[t]
                        if (half, h) not in state:
                            state[(half, h)] = pso.tile([Hd, HW], f32, name="ps_out")
                        ps_out = state[(half, h)]
                        probs = probs_by_idx.pop(t)
                        st = dict(start=(tk == 0), stop=(tk == MT - 1))
                        for j in range(2):
                            nc.tensor.matmul(
                                ps_out[:, j * 512 : (j + 1) * 512],
                                v_big[:, tk, h * 128 : (h + 1) * 128],
                                probs[:, j * 512 : (j + 1) * 512],
                                **st,
                            )
                        # denominator: accumulate probs tiles on the DVE into
                        # 3 bf16 partial sums; GpSimd pre-reduces those to
                        # one, so the PE only does 2 single-shot ones-matmuls
                        # per unit. bf16 partial rounding adds ~0.2% den
                        # error.
                        p_idx = P_OF_TK[tk]
                        parts = partials.setdefault((half, h), [None] * 3)
                        if parts[p_idx] is None:
                            pt = att.tile(
                                [128, HW], bf16, name=f"part{p_idx}", bufs=2
                            )
                            parts[p_idx] = pt
                            nc.vector.tensor_copy(pt, probs)
                        else:
                            pt = parts[p_idx]
                            nc.vector.tensor_add(pt, pt, probs)
                        if tk == 13:
                            # p0 += p1 early (p1 completed at tk==10)
                            nc.gpsimd.tensor_add(parts[0], parts[0], parts[1])

                    def tail_a(t):
                        half, h, tk = seq[t]
                        ps_out = state[(half, h)]
                        # copy out unnormalized attention output, and finish
                        # the partial pre-reduce off the PE
                        sl = slice(half * HW, (half + 1) * HW)
                        nc.vector.tensor_copy(outT[h][:, sl], ps_out)
                        parts = partials[(half, h)]
                        nc.gpsimd.tensor_add(parts[0], parts[0], parts[2])

                    def tail_b(t):
                        half, h, tk = seq[t]
                        ps_out = state.pop((half, h))
                        parts = partials.pop((half, h))
                        # denominator row: 2 single-shot ones-matmuls into
                        # ps_out[0:1] (bank already copied out by tail_a, so
                        # no dedicated PSUM banks for the denominator)
                        den_ap = ps_out[0:1, :]
                        for j in range(2):
                            nc.tensor.matmul(
                                den_ap[:, j * 512 : (j + 1) * 512],
                                ones_b,
                                parts[0][:, j * 512 : (j + 1) * 512],
                                start=True,
                                stop=True,
                            )
                        # fast reciprocal on DVE, partition broadcast on
                        # GpSimd, in-place DVE normalize
                        sl = slice(half * HW, (half + 1) * HW)
                        rec = att.tile([1, HW], f32, name="rec", bufs=2)
                        nc.vector.reciprocal_approx_fast(rec, den_ap)
                        rec_bc = att.tile([128, HW], f32, name="rec_bc", bufs=2)
                        nc.gpsimd.partition_broadcast(rec_bc, rec, channels=128)
                        nc.vector.tensor_mul(
                            outT[h][:, sl], outT[h][:, sl], rec_bc
                        )

                    # ---- o_proj work quanta -------------------------------
                    # one quantum = one (m, nblk-pair): 2 accumulation chains
                    # of 4 matmuls each, alternating between the 2 banks of
                    # the single pof tile, + 2 PSUM->SBUF copies; DMA per m.
                    ot4_by_m = {}
                    oproj_copy_n = [0]

                    def oproj_quantum(m, pair, engines):
                        of = pof.tile([128, HW], f32, name="of")
                        if pair == 0:
                            ot4_by_m[m] = att.tile(
                                [128, 4, 512], bf16, name="ot4", bufs=3
                            )
                        ot4 = ot4_by_m[m]
                        for nblk in (2 * pair, 2 * pair + 1):
                            bsl = slice((nblk % 2) * 512, (nblk % 2) * 512 + 512)
                            for hh in range(TPH):
                                nc.tensor.matmul(
                                    of[:, bsl],
                                    outT[hh][:, m * 128 : (m + 1) * 128],
                                    wo_sb[:, hh, nblk * 512 : (nblk + 1) * 512],
                                    start=(hh == 0),
                                    stop=(hh == TPH - 1),
                                )
                        for nblk in (2 * pair, 2 * pair + 1):
                            bsl = slice((nblk % 2) * 512, (nblk % 2) * 512 + 512)
                            eng = engines[oproj_copy_n[0] % len(engines)]
                            oproj_copy_n[0] += 1
                            if eng == "v":
                                nc.vector.tensor_copy(ot4[:, nblk, :], of[:, bsl])
                            elif eng == "s":
                                nc.scalar.copy(ot4[:, nblk, :], of[:, bsl])
                            else:
                                nc.gpsimd.tensor_copy(ot4[:, nblk, :], of[:, bsl])
                        if pair == 1:
                            ot4 = ot4_by_m.pop(m)
                            if m == MT - 1:
                                # last tile: per-d-block DMAs to shorten the
                                # final drain
                                for nblk in range(4):
                                    nc.sync.dma_start(
                                        out=out_re[:, m, nblk, :],
                                        in_=ot4[:, nblk, :],
                                    )
                            else:
                                nc.sync.dma_start(out=out_re[:, m, :, :], in_=ot4)

                    oproj_work = [(m, pair) for m in range(MT) for pair in range(2)]
                    oproj_next = [0]

                    def emit_oproj(engines):
                        if oproj_next[0] < len(oproj_work):
                            m, pair = oproj_work[oproj_next[0]]
                            oproj_next[0] += 1
                            oproj_quantum(m, pair, engines)

                    # interleave a few o_proj quanta into the second half of
                    # the attention stream (half-0 outT is final there and
                    # the PE has slack while ACT streams exps)
                    OPROJ_STEPS = {72, 84, 96, 108, 120}

                    # 2-wide supersteps: sc pairs for (s, s+1) then av pairs
                    # for (s-2, s-1); unit tails split so the denominator
                    # matmuls never delay the next unit's score matmuls
                    # (which feed the ACT exp stream).
                    for s in range(0, n_seq + PIPE, 2):
                        for t in (s, s + 1):
                            if t < n_seq:
                                front(t)
                        for t in (s - PIPE, s - PIPE + 1):
                            if 0 <= t < n_seq:
                                back_av(t)
                        for t in (s - PIPE, s - PIPE + 1):
                            if 0 <= t < n_seq and seq[t][2] == MT - 1:
                                tail_a(t)
                                tail_b(t)
                        if s in OPROJ_STEPS:
                            emit_oproj(("v",))
                    # rest of o_proj; copies go ACT/DVE now that the exp
                    # stream is done
                    while oproj_next[0] < len(oproj_work):
                        emit_oproj(("s", "v"))

    nc.compile()
    return nc


def _bf(x: np.ndarray) -> np.ndarray:
    return np.ascontiguousarray(x, dtype=np.float32).astype(ml_dtypes.bfloat16)


def kernel(hidden_states, cos, sin, wq, wk, wv, wo):
    if "nc" not in _CACHE:
        _CACHE["nc"] = _build()
    nc = _CACHE["nc"]

    hidden_states = np.asarray(hidden_states, dtype=np.float32)
    cos = np.asarray(cos, dtype=np.float32)
    sin = np.asarray(sin, dtype=np.float32)
    wq = np.asarray(wq, dtype=np.float32)
    wk = np.asarray(wk, dtype=np.float32)
    wv = np.asarray(wv, dtype=np.float32)
    wo = np.asarray(wo, dtype=np.float32)

    # host-side layout prep
    cosT = _bf(cos[0, 0].T)                             # [Hd, L]
    sinT = np.ascontiguousarray(sin[0, 0].T)            # [Hd, L]
    sinTs = sinT.copy()
    sinTs[: Hd // 2] *= -1.0                            # fold rotate_half signs
    sinTs = _bf(sinTs)

    # pre-tile for contiguous per-partition DMA lines:
    #   hTt[n, p, kk, t] = h.T[kk*128+p, n*512+t]
    #   w*T[p, kk, r]    = w[r, kk*128+p]   (w.T row d = kk*128+p)
    #   woT[p, hh, d]    = wo[d, r0+hh*128+p]
    def _tile_h(hb):
        return _bf(
            hb.T.reshape(KT, 128, NT, 512).transpose(2, 1, 0, 3)
        )

    def _tile_w(wrows):
        return _bf(wrows.T.reshape(KT, 128, QKV).transpose(1, 0, 2))

    def _tile_wo(wcols):
        return _bf(wcols.T.reshape(TPH, 128, D).transpose(1, 0, 2))

    hTt = [_tile_h(hidden_states[b]) for b in range(B)]
    # fold the attention scale into wq so raw scores are ~N(0,1) (keeps the
    # bf16 PSUM score rounding relative) and exp() needs no scale
    wq = wq * SCALE

    in_maps = []
    for c in range(NC):
        b = c // 4
        hb = c % 4
        r0 = hb * QKV
        in_maps.append(
            {
                "hTt": hTt[b],
                "wqT": _tile_w(wq[r0 : r0 + QKV]),
                "wkT": _tile_w(wk[r0 : r0 + QKV]),
                "wvT": _tile_w(wv[r0 : r0 + QKV]),
                "woT": _tile_wo(wo[:, r0 : r0 + QKV]),
                "cosT": cosT,
                "sinTs": sinTs,
            }
        )

    res = run_bass_kernel_spmd(nc, in_maps, core_ids=list(range(NC)))
    _CACHE["last_results"] = res

    out = np.zeros((B, L, D), dtype=np.float32)
    for c in range(NC):
        out[c // 4] += np.asarray(res.results[c]["out"], dtype=np.float32)
    return out


# revision 12
# speedup vs baseline: 1.0972x; 1.0972x over previous
"""Multi-head attention layer (QKV proj + RoPE + SDPA + o_proj) on 8 TRN2 cores.

Sharding: DP2 x TP4. Core c handles batch c//4 and heads 4*(c%4)..4*(c%4)+4.
Each core computes its 4 heads' attention and a partial o_proj output
[L, D]; the host sums the 4 partials per batch (row-parallel o_proj).

All matmul operands are bf16 (same 1 cycle/row PE rate as fp32r on TRN2,
half the DMA/SBUF footprint); PSUM accumulation is fp32 throughout.

Structure (single PE stream, minimal gaps):
  phase 1: for each 512-token block n, stream htk once and run q, k, v
           projections back-to-back out of the same SBUF tiles. RoPE is
           PE-free: the rotate-half permutation is done with two ACT
           partition-offset copies (sign folded into sin host-side), the
           cos/sin multiplies run on DVE in bf16 (2x mode), and the final
           add runs on GpSimd. Rope work for block n's flush is spread
           into the following projection stream (one unit per kg
           boundary) so no engine queue piles up.
  phase 2: attention, 2-kv-tile supersteps with a 2-step software
           pipeline: scores for tiles (t, t+1) are issued before av(t-2),
           av(t-1), so the exp (ACT) latency hides behind PE work. Scores
           land in a [128,1024] fp32 PSUM tile (two single-shot 512-wide
           matmuls -> one 1024-wide exp; wq carries the 1/sqrt(Hd)
           scale). Softmax skips max-subtraction (scores ~N(0,1)).
           Denominator: probs accumulate into 3 bf16 partials on DVE
           (GpSimd takes the early pre-reduce, DVE the final combine);
           the PE only does 2 single-shot ones-matmuls per (head, half),
           written into ps_out[0:1] after its copy-out (so no PSUM banks
           are reserved for the denominator). ps_out is double-buffered
           and the den matmuls are emitted two supersteps late, so
           neither the in-order PE queue nor the next unit's av chain
           ever waits on the tail. Reciprocal on DVE, broadcast across
           partitions with gpsimd.partition_broadcast (no DRAM bounce),
           in-place DVE normalize.
  phase 3: o_proj lives in the same PSUM pool scope as attention: each
           (token-tile, d-block-pair) quantum runs its two 4-matmul
           accumulation chains in a PSUM slot borrowed round-robin from
           the attention pools (8 banks in rotation), starting while the
           last attention tails drain. bf16 output DMA (summed to fp32 on
           the host); the last token tile DMAs per-d-block to shorten the
           drain.

Accumulation-chain rule learned the hard way: `start=True` clears the
has_written bits for the WHOLE PSUM bank, so two interleaved multi-step
accumulation chains must never share a bank (single-shot matmuls may).
"""

import numpy as np

import sys
import types

# Defensive: concourse.bass_utils imports antenv.axon_hooks when tracing is
# requested; provide a null shim if the module is absent in this image so a
# stray BASS_TRACE env var cannot crash the kernel.
try:
    import antenv.axon_hooks  # noqa: F401
except ImportError:
    _m = types.ModuleType("antenv.axon_hooks")
    _m.set_axon_ntff_profile_hook = lambda h: None
    _m.get_axon_ntff_profile_hook = lambda: None
    sys.modules["antenv.axon_hooks"] = _m

import ml_dtypes

import concourse.bass as bass
import concourse.mybir as mybir
import concourse.tile as tile
from concourse import bacc
from concourse.bass_utils import run_bass_kernel_spmd

# problem constants (hardcoded per spec)
B, L, D = 2, 2048, 2048
H, Hd = 16, 128
NC = 8
TPH = 4            # heads per core
QKV = TPH * Hd     # 512 per-core projection width
KT = D // 128      # 16 contraction tiles
NT = L // 512      # 4 token groups of 512
MT = L // 128      # 16 token chunks of 128

f32 = mybir.dt.float32
bf16 = mybir.dt.bfloat16

AF = mybir.ActivationFunctionType
SCALE = 1.0 / float(np.sqrt(Hd))

_CACHE: dict = {}


def _build():
    nc = bacc.Bacc("TRN2", target_bir_lowering=False, debug=False)

    # inputs are pre-tiled on the host so every DMA line is contiguous per
    # partition (4-16KB instead of 1KB)
    hTt = nc.dram_tensor("hTt", [NT, 128, KT, 512], bf16, kind="ExternalInput").ap()
    wqT = nc.dram_tensor("wqT", [128, KT, QKV], bf16, kind="ExternalInput").ap()
    wkT = nc.dram_tensor("wkT", [128, KT, QKV], bf16, kind="ExternalInput").ap()
    wvT = nc.dram_tensor("wvT", [128, KT, QKV], bf16, kind="ExternalInput").ap()
    woT = nc.dram_tensor("woT", [128, TPH, D], bf16, kind="ExternalInput").ap()
    cosT = nc.dram_tensor("cosT", [Hd, L], bf16, kind="ExternalInput").ap()
    sinTs = nc.dram_tensor("sinTs", [Hd, L], bf16, kind="ExternalInput").ap()
    out = nc.dram_tensor("out", [L, D], bf16, kind="ExternalOutput").ap()

    out_re = out.rearrange("(mm p) (nb d) -> p mm nb d", p=128, d=512)

    with tile.TileContext(nc) as tc:
        with tc.tile_pool(name="persist", bufs=1) as persist:
            # ---- persistent tensors -----------------------------------
            ones_b = persist.tile([128, 1], bf16, name="ones_b")
            nc.vector.memset(ones_b, 1.0)
            warm = persist.tile([128, 512], bf16, name="warm")
            nc.vector.memset(warm, 0.0)
            qT = [persist.tile([Hd, L], bf16, name=f"qT{h}") for h in range(TPH)]
            kT = [persist.tile([Hd, L], bf16, name=f"kT{h}") for h in range(TPH)]
            v_big = persist.tile([128, MT, QKV], bf16, name="v_big")
            outT = [persist.tile([Hd, L], bf16, name=f"outT{h}") for h in range(TPH)]
            cos_sb = persist.tile([Hd, L], bf16, name="cos_sb")
            sin_sb = persist.tile([Hd, L], bf16, name="sin_sb")

            # ==== phase 1: fused q/k/v projections + RoPE ====================
            with (
                tc.tile_pool(name="wqkv", bufs=1) as wqkv,
                tc.tile_pool(name="stream", bufs=2) as stream,
                tc.tile_pool(name="tmp", bufs=2) as tmp,
                tc.tile_pool(name="pp", bufs=1, space="PSUM") as pp,
            ):
                # first htk block ahead of everything else on the sync queue;
                # leading chunks are fine-grained so the first matmul can
                # start as soon as ktile 0 and wq ktile 0 land
                htk0 = stream.tile([128, KT, 512], bf16, name="htk")
                for kg in range(4):
                    nc.sync.dma_start(
                        out=htk0[:, kg * 4 : (kg + 1) * 4, :],
                        in_=hTt[0, :, kg * 4 : (kg + 1) * 4, :],
                    )
                # weight prefetch, tag-major: q is consumed first (+4us),
                # then k (+17us), then v (+31us)
                w_sbs = {}
                for tag in ("q", "k", "v"):
                    w_sbs[tag] = wqkv.tile([128, KT, QKV], bf16, name=f"w_{tag}")
                w_res = {"q": wqT, "k": wkT, "v": wvT}
                for tag in ("q", "k", "v"):
                    w_re = w_res[tag]
                    for a, b in (
                        (0, 1), (1, 2), (2, 4), (4, 6), (6, 8),
                        (8, 10), (10, 12), (12, 16),
                    ):
                        nc.gpsimd.dma_start(
                            out=w_sbs[tag][:, a:b, :], in_=w_re[:, a:b, :]
                        )
                # rope constants: needed at the first flush (~14us in)
                nc.sync.dma_start(out=cos_sb, in_=cosT)
                nc.sync.dma_start(out=sin_sb, in_=sinTs)

                # PE warm-up: junk matmuls during the initial DMA wait so the
                # clock ramp (free-running ~3.4us activity window) completes
                # before real data lands
                ps_warm = pp.tile([128, 512], f32, name="pp0", bufs=2)
                for _ in range(7):
                    nc.tensor.matmul(
                        ps_warm, warm[:, 0:128], warm, start=True, stop=True
                    )

                # deferred rope work: list of closures, emitted one per kg
                # boundary of the *following* projection stream so the ACT
                # queue (which also drains the PSUM flush copies) never
                # backs up.
                pending_rope = []

                def emit_one_rope():
                    if pending_rope:
                        pending_rope.pop(0)()

                def make_rope(dst, m, n, raw):
                    csl = slice(n * 512, (n + 1) * 512)

                    def do():
                        # rotate-half permutation: two partition-offset
                        # copies on ACT (sign already folded into sinTs)
                        rr = tmp.tile([128, 512], bf16, name="rr")
                        nc.scalar.copy(rr[0:64, :], raw[64:128, :])
                        nc.scalar.copy(rr[64:128, :], raw[0:64, :])
                        t1 = tmp.tile([128, 512], bf16, name="t1")
                        nc.vector.tensor_mul(t1, raw, cos_sb[:, csl])
                        t2 = tmp.tile([128, 512], bf16, name="t2")
                        nc.vector.tensor_mul(t2, rr, sin_sb[:, csl])
                        nc.gpsimd.tensor_add(dst[m][:, csl], t1, t2)

                    return do

                for n in range(NT):
                    if n == 0:
                        htk = htk0
                    else:
                        htk = stream.tile([128, KT, 512], bf16, name="htk")
                        for kg in range(2):
                            nc.sync.dma_start(
                                out=htk[:, kg * 8 : (kg + 1) * 8, :],
                                in_=hTt[n, :, kg * 8 : (kg + 1) * 8, :],
                            )
                    for tag, dst in (("q", qT), ("k", kT), ("v", None)):
                        w_sb = w_sbs[tag]
                        ps_x = [
                            pp.tile(
                                [128, 512], f32, name=f"pp{m}",
                                bufs=2 if m < 2 else 1,
                            )
                            for m in range(TPH)
                        ]
                        if tag == "v":
                            # v: stationary = token columns of htk, moving = wv
                            # rows; out [token128, qkv512] per token sub-tile.
                            for kg in range(4):
                                for mc in range(4):
                                    for i in range(4):
                                        kk = kg * 4 + i
                                        nc.tensor.matmul(
                                            ps_x[mc],
                                            htk[:, kk, mc * 128 : (mc + 1) * 128],
                                            w_sb[:, kk, :],
                                            start=(kk == 0),
                                            stop=(kk == KT - 1),
                                        )
                                emit_one_rope()
                            for mc in range(4):
                                if mc < 2:
                                    nc.scalar.copy(v_big[:, n * 4 + mc, :], ps_x[mc])
                                else:
                                    nc.vector.tensor_copy(
                                        v_big[:, n * 4 + mc, :], ps_x[mc]
                                    )
                        else:
                            # q/k: stationary = weight m-tile, moving = htk.
                            # m-major within each kg so the first matmul of
                            # m2/m3 (single-buffered banks) comes late enough
                            # for the previous flush to have freed them.
                            for kg in range(4):
                                for m in range(TPH):
                                    for i in range(4):
                                        kk = kg * 4 + i
                                        nc.tensor.matmul(
                                            ps_x[m],
                                            w_sb[:, kk, m * 128 : (m + 1) * 128],
                                            htk[:, kk, :],
                                            start=(kk == 0),
                                            stop=(kk == KT - 1),
                                        )
                                emit_one_rope()
                            # flush: raw copies split ACT/DVE, rope deferred
                            raws = []
                            for m in range(TPH):
                                raw = tmp.tile([128, 512], bf16, name="raw", bufs=4)
                                if m < 2:
                                    nc.scalar.copy(raw, ps_x[m])
                                else:
                                    nc.vector.tensor_copy(raw, ps_x[m])
                                raws.append(raw)
                            for m in range(TPH):
                                pending_rope.append(make_rope(dst, m, n, raws[m]))
                while pending_rope:
                    emit_one_rope()

            # ==== phase 2+3: attention with fused o_proj =====================
            with (
                tc.tile_pool(name="wo", bufs=1) as wop,
                tc.tile_pool(name="att", bufs=2) as att,
            ):
                wo_sb = wop.tile([128, TPH, D], bf16, name="wo_sb")
                for hh in range(TPH):
                    nc.gpsimd.dma_start(out=wo_sb[:, hh, :], in_=woT[:, hh, :])

                HW = 1024  # tq half-width

                seq = [
                    (half, h, tk)
                    for half in range(2)
                    for h in range(TPH)
                    for tk in range(MT)
                ]
                n_seq = len(seq)
                state = {}  # (half,h) -> ps_out
                probs_by_idx = {}
                partials = {}
                # partial index by tk: p0 = tk0-5, p1 = tk6-10, p2 = tk11-15
                P_OF_TK = [0] * 6 + [1] * 5 + [2] * 5

                PIPE = 2  # av trails scores by 2 steps so exp() is done
                with (
                    tc.tile_pool(name="pss", bufs=2, space="PSUM") as pss,
                    tc.tile_pool(name="pso", bufs=2, space="PSUM") as pso,
                ):
                    def front(t):
                        half, h, tk = seq[t]
                        # full-width scores tile (2 banks); each 512-wide
                        # matmul is single-shot (start&stop) into its own
                        # bank. One 1024-wide exp serves both. wq is
                        # pre-scaled by 1/sqrt(Hd) on the host.
                        sc_ps = pss.tile([128, HW], f32, name="sc")
                        for j in range(2):
                            tq0 = half * HW + j * 512
                            nc.tensor.matmul(
                                sc_ps[:, j * 512 : (j + 1) * 512],
                                kT[h][:, tk * 128 : (tk + 1) * 128],
                                qT[h][:, tq0 : tq0 + 512],
                                start=True,
                                stop=True,
                            )
                        probs = att.tile([128, HW], bf16, name="probs", bufs=5)
                        probs_by_idx[t] = probs
                        nc.scalar.activation(probs, sc_ps, AF.Exp)

                    def back_av(t):
                        half, h, tk = seq[t]
                        if (half, h) not in state:
                            state[(half, h)] = pso.tile([Hd, HW], f32, name="ps_out")
                        ps_out = state[(half, h)]
                        probs = probs_by_idx.pop(t)
                        st = dict(start=(tk == 0), stop=(tk == MT - 1))
                        for j in range(2):
                            nc.tensor.matmul(
                                ps_out[:, j * 512 : (j + 1) * 512],
                                v_big[:, tk, h * 128 : (h + 1) * 128],
                                probs[:, j * 512 : (j + 1) * 512],
                                **st,
                            )
                        # denominator: accumulate probs tiles on the DVE into
                        # 3 bf16 partial sums; GpSimd pre-reduces those to
                        # one, so the PE only does 2 single-shot ones-matmuls
                        # per unit. bf16 partial rounding adds ~0.2% den
                        # error.
                        p_idx = P_OF_TK[tk]
                        parts = partials.setdefault((half, h), [None] * 3)
                        if parts[p_idx] is None:
                            pt = att.tile(
                                [128, HW], bf16, name=f"part{p_idx}", bufs=2
                            )
                            parts[p_idx] = pt
                            nc.vector.tensor_copy(pt, probs)
                        else:
                            pt = parts[p_idx]
                            nc.vector.tensor_add(pt, pt, probs)
                        if tk == 13:
                            # p0 += p1 early (p1 completed at tk==10); runs
                            # on GpSimd, off every critical path
                            nc.gpsimd.tensor_add(parts[0], parts[0], parts[1])

                    def tail_a(t):
                        half, h, tk = seq[t]
                        ps_out = state[(half, h)]
                        # copy out unnormalized attention output, and finish
                        # the partial pre-reduce; both on DVE so the final
                        # combine has no cross-engine latency behind it
                        sl = slice(half * HW, (half + 1) * HW)
                        nc.vector.tensor_copy(outT[h][:, sl], ps_out)
                        parts = partials[(half, h)]
                        nc.vector.tensor_add(parts[0], parts[0], parts[2])

                    def tail_b(t):
                        half, h, tk = seq[t]
                        ps_out = state.pop((half, h))
                        parts = partials.pop((half, h))
                        # denominator row: 2 single-shot ones-matmuls into
                        # ps_out[0:1] (bank already copied out by tail_a, so
                        # no dedicated PSUM banks for the denominator)
                        den_ap = ps_out[0:1, :]
                        for j in range(2):
                            nc.tensor.matmul(
                                den_ap[:, j * 512 : (j + 1) * 512],
                                ones_b,
                                parts[0][:, j * 512 : (j + 1) * 512],
                                start=True,
                                stop=True,
                            )
                        # fast reciprocal on DVE, partition broadcast on
                        # GpSimd, in-place DVE normalize
                        sl = slice(half * HW, (half + 1) * HW)
                        rec = att.tile([1, HW], f32, name="rec", bufs=2)
                        nc.vector.reciprocal_approx_fast(rec, den_ap)
                        rec_bc = att.tile([128, HW], f32, name="rec_bc", bufs=2)
                        nc.gpsimd.partition_broadcast(rec_bc, rec, channels=128)
                        nc.vector.tensor_mul(
                            outT[h][:, sl], outT[h][:, sl], rec_bc
                        )

                    # ---- o_proj work quanta -------------------------------
                    # one quantum = one (m, nblk-pair): 2 accumulation chains
                    # of 4 matmuls each, in the 2 banks of a PSUM slot
                    # borrowed from the attention pools (rotating over all 4
                    # slot-groups = 8 banks, so chains never wait on the
                    # previous quantum's copies), + 2 PSUM->SBUF copies; DMA
                    # per m.
                    ot4_by_m = {}
                    oproj_copy_n = [0]

                    def oproj_quantum(m, pair, engines, pool, tname):
                        of = pool.tile([128, HW], f32, name=tname)
                        if pair == 0:
                            ot4_by_m[m] = att.tile(
                                [128, 4, 512], bf16, name="ot4", bufs=3
                            )
                        ot4 = ot4_by_m[m]
                        for nblk in (2 * pair, 2 * pair + 1):
                            bsl = slice((nblk % 2) * 512, (nblk % 2) * 512 + 512)
                            for hh in range(TPH):
                                nc.tensor.matmul(
                                    of[:, bsl],
                                    outT[hh][:, m * 128 : (m + 1) * 128],
                                    wo_sb[:, hh, nblk * 512 : (nblk + 1) * 512],
                                    start=(hh == 0),
                                    stop=(hh == TPH - 1),
                                )
                        for nblk in (2 * pair, 2 * pair + 1):
                            bsl = slice((nblk % 2) * 512, (nblk % 2) * 512 + 512)
                            eng = engines[oproj_copy_n[0] % len(engines)]
                            oproj_copy_n[0] += 1
                            if eng == "v":
                                nc.vector.tensor_copy(ot4[:, nblk, :], of[:, bsl])
                            elif eng == "s":
                                nc.scalar.copy(ot4[:, nblk, :], of[:, bsl])
                            else:
                                nc.gpsimd.tensor_copy(ot4[:, nblk, :], of[:, bsl])
                        if pair == 1:
                            ot4 = ot4_by_m.pop(m)
                            if m == MT - 1:
                                # last tile: per-d-block DMAs to shorten the
                                # final drain
                                for nblk in range(4):
                                    nc.sync.dma_start(
                                        out=out_re[:, m, nblk, :],
                                        in_=ot4[:, nblk, :],
                                    )
                            else:
                                nc.sync.dma_start(out=out_re[:, m, :, :], in_=ot4)

                    # 2-wide supersteps: sc pairs for (s, s+1) then av pairs
                    # for (s-2, s-1). A unit's tail is split: tail_a (DVE
                    # copy-out + final partial combine) runs with its
                    # superstep; tail_b (den matmuls + recip + broadcast +
                    # normalize) is delayed one superstep so the den matmul
                    # reaches the front of the in-order PE queue only after
                    # its DVE dependencies have resolved. ps_out is
                    # double-buffered, so the next unit accumulates into the
                    # other slot while the tail chain drains.
                    pending_tail_b = []  # (due_superstep, t)
                    for s in range(0, n_seq + 8, 2):
                        for t in (s, s + 1):
                            if t < n_seq:
                                front(t)
                        for t in (s - PIPE, s - PIPE + 1):
                            if 0 <= t < n_seq:
                                back_av(t)
                        while pending_tail_b and pending_tail_b[0][0] <= s:
                            tail_b(pending_tail_b.pop(0)[1])
                        for t in (s - PIPE, s - PIPE + 1):
                            if 0 <= t < n_seq and seq[t][2] == MT - 1:
                                tail_a(t)
                                pending_tail_b.append((s + 4, t))
                    # o_proj: rotate quanta over the four freed PSUM slot
                    # groups; copies alternate ACT/DVE now that the exp
                    # stream is done
                    slot_cycle = [(pss, "sc"), (pss, "sc"), (pso, "ps_out"), (pso, "ps_out")]
                    qn = 0
                    for m in range(MT):
                        for pair in range(2):
                            pool, tname = slot_cycle[qn % 4]
                            qn += 1
                            oproj_quantum(m, pair, ("s", "v"), pool, tname)

    nc.compile()
    return nc


def _bf(x: np.ndarray) -> np.ndarray:
    return np.ascontiguousarray(x, dtype=np.float32).astype(ml_dtypes.bfloat16)


def kernel(hidden_states, cos, sin, wq, wk, wv, wo):
    if "nc" not in _CACHE:
        _CACHE["nc"] = _build()
    nc = _CACHE["nc"]

    hidden_states = np.asarray(hidden_states, dtype=np.float32)
    cos = np.asarray(cos, dtype=np.float32)
    sin = np.asarray(sin, dtype=np.float32)
    wq = np.asarray(wq, dtype=np.float32)
    wk = np.asarray(wk, dtype=np.float32)
    wv = np.asarray(wv, dtype=np.float32)
    wo = np.asarray(wo, dtype=np.float32)

    # host-side layout prep
    cosT = _bf(cos[0, 0].T)                             # [Hd, L]
    sinT = np.ascontiguousarray(sin[0, 0].T)            # [Hd, L]
    sinTs = sinT.copy()
    sinTs[: Hd // 2] *= -1.0                            # fold rotate_half signs
    sinTs = _bf(sinTs)

    # pre-tile for contiguous per-partition DMA lines:
    #   hTt[n, p, kk, t] = h.T[kk*128+p, n*512+t]
    #   w*T[p, kk, r]    = w[r, kk*128+p]   (w.T row d = kk*128+p)
    #   woT[p, hh, d]    = wo[d, r0+hh*128+p]
    def _tile_h(hb):
        return _bf(
            hb.T.reshape(KT, 128, NT, 512).transpose(2, 1, 0, 3)
        )

    def _tile_w(wrows):
        return _bf(wrows.T.reshape(KT, 128, QKV).transpose(1, 0, 2))

    def _tile_wo(wcols):
        return _bf(wcols.T.reshape(TPH, 128, D).transpose(1, 0, 2))

    hTt = [_tile_h(hidden_states[b]) for b in range(B)]
    # fold the attention scale into wq so raw scores are ~N(0,1) (keeps the
    # bf16 PSUM score rounding relative) and exp() needs no scale
    wq = wq * SCALE

    in_maps = []
    for c in range(NC):
        b = c // 4
        hb = c % 4
        r0 = hb * QKV
        in_maps.append(
            {
                "hTt": hTt[b],
                "wqT": _tile_w(wq[r0 : r0 + QKV]),
                "wkT": _tile_w(wk[r0 : r0 + QKV]),
                "wvT": _tile_w(wv[r0 : r0 + QKV]),
                "woT": _tile_wo(wo[:, r0 : r0 + QKV]),
                "cosT": cosT,
                "sinTs": sinTs,
            }
        )

    res = run_bass_kernel_spmd(nc, in_maps, core_ids=list(range(NC)))
    _CACHE["last_results"] = res

    out = np.zeros((B, L, D), dtype=np.float32)
    for c in range(NC):
        out[c // 4] += np.asarray(res.results[c]["out"], dtype=np.float32)
    return out


# revision 16
# speedup vs baseline: 1.2874x; 1.1734x over previous
"""Multi-head attention layer (QKV proj + RoPE + SDPA + o_proj) on 8 TRN2 cores.

Sharding: DP2 x TP4. Core c handles batch c//4 and heads 4*(c%4)..4*(c%4)+4.
Each core computes its 4 heads' attention and a partial o_proj output
[L, D]; the host sums the 4 partials per batch (row-parallel o_proj).

All matmul operands are bf16 (same 1 cycle/row PE rate as fp32r on TRN2,
half the DMA/SBUF footprint); PSUM accumulation is fp32 throughout.

Structure (single PE stream, minimal gaps):
  phase 1: for each 512-token block n, stream htk once and run q, k, v
           projections back-to-back out of the same SBUF tiles. RoPE is
           PE-free: the rotate-half permutation is done with two ACT
           partition-offset copies (sign folded into sin host-side), the
           cos/sin multiplies run on DVE in bf16 (2x mode), and the final
           add runs on GpSimd. Rope work for block n's flush is spread
           into the following projection stream (one unit per kg
           boundary) so no engine queue piles up.
  phase 2: attention, 2-kv-tile supersteps with a 2-step software
           pipeline: scores for tiles (t, t+1) are issued before av(t-2),
           av(t-1), so the exp (ACT) latency hides behind PE work. Scores
           land in a [128,1024] fp32 PSUM tile (two single-shot 512-wide
           matmuls -> one 1024-wide exp; wq carries the 1/sqrt(Hd)
           scale). Softmax skips max-subtraction (scores ~N(0,1)).
           Denominator: probs accumulate into 2 bf16 partials on DVE
           (combined on DVE at unit end — never a GpSimd hop, whose
           multi-us op latency would gate the in-order PE queue);
           the PE only does 2 single-shot ones-matmuls per (head, half),
           written into ps_out[0:1] after its copy-out (so no PSUM banks
           are reserved for the denominator). ps_out is double-buffered
           and the den matmuls are emitted two supersteps late, so
           neither the in-order PE queue nor the next unit's av chain
           ever waits on the tail. Reciprocal on DVE, broadcast across
           partitions with gpsimd.partition_broadcast (no DRAM bounce),
           in-place DVE normalize.
  phase 3: o_proj lives in the same PSUM pool scope as attention: each
           (token-tile, d-block-pair) quantum runs its two 4-matmul
           accumulation chains in a PSUM slot borrowed round-robin from
           the attention pools (8 banks in rotation), starting while the
           last attention tails drain. bf16 output DMA (summed to fp32 on
           the host); the last token tile DMAs per-d-block to shorten the
           drain.

Accumulation-chain rule learned the hard way: `start=True` clears the
has_written bits for the WHOLE PSUM bank, so two interleaved multi-step
accumulation chains must never share a bank (single-shot matmuls may).
"""

import numpy as np

import sys
import types

# Defensive: concourse.bass_utils imports antenv.axon_hooks when tracing is
# requested; provide a null shim if the module is absent in this image so a
# stray BASS_TRACE env var cannot crash the kernel.
try:
    import antenv.axon_hooks  # noqa: F401
except ImportError:
    _m = types.ModuleType("antenv.axon_hooks")
    _m.set_axon_ntff_profile_hook = lambda h: None
    _m.get_axon_ntff_profile_hook = lambda: None
    sys.modules["antenv.axon_hooks"] = _m

import ml_dtypes

import concourse.bass as bass
import concourse.mybir as mybir
import concourse.tile as tile
from concourse import bacc
from concourse.bass_utils import run_bass_kernel_spmd

# problem constants (hardcoded per spec)
B, L, D = 2, 2048, 2048
H, Hd = 16, 128
NC = 8
TPH = 4            # heads per core
QKV = TPH * Hd     # 512 per-core projection width
KT = D // 128      # 16 contraction tiles
NT = L // 512      # 4 token groups of 512
MT = L // 128      # 16 token chunks of 128

f32 = mybir.dt.float32
bf16 = mybir.dt.bfloat16

AF = mybir.ActivationFunctionType
SCALE = 1.0 / float(np.sqrt(Hd))

_CACHE: dict = {}


def _build():
    nc = bacc.Bacc("TRN2", target_bir_lowering=False, debug=False)

    # inputs are pre-tiled on the host so every DMA line is contiguous per
    # partition (4-16KB instead of 1KB)
    hTt = nc.dram_tensor("hTt", [NT, 128, KT, 512], bf16, kind="ExternalInput").ap()
    wqT = nc.dram_tensor("wqT", [128, KT, QKV], bf16, kind="ExternalInput").ap()
    wkT = nc.dram_tensor("wkT", [128, KT, QKV], bf16, kind="ExternalInput").ap()
    wvT = nc.dram_tensor("wvT", [128, KT, QKV], bf16, kind="ExternalInput").ap()
    woT = nc.dram_tensor("woT", [128, TPH, D], bf16, kind="ExternalInput").ap()
    cosT = nc.dram_tensor("cosT", [Hd, L], bf16, kind="ExternalInput").ap()
    sinTs = nc.dram_tensor("sinTs", [Hd, L], bf16, kind="ExternalInput").ap()
    out = nc.dram_tensor("out", [L, D], bf16, kind="ExternalOutput").ap()

    out_re = out.rearrange("(mm p) (nb d) -> p mm nb d", p=128, d=512)

    with tile.TileContext(nc) as tc:
        with tc.tile_pool(name="persist", bufs=1) as persist:
            # ---- persistent tensors -----------------------------------
            ones_b = persist.tile([128, 1], bf16, name="ones_b")
            nc.vector.memset(ones_b, 1.0)
            warm = persist.tile([128, 512], bf16, name="warm")
            nc.vector.memset(warm, 0.0)
            qT = [persist.tile([Hd, L], bf16, name=f"qT{h}") for h in range(TPH)]
            kT = [persist.tile([Hd, L], bf16, name=f"kT{h}") for h in range(TPH)]
            v_big = persist.tile([128, MT, QKV], bf16, name="v_big")
            outT = [persist.tile([Hd, L], bf16, name=f"outT{h}") for h in range(TPH)]
            cos_sb = persist.tile([Hd, L], bf16, name="cos_sb")
            sin_sb = persist.tile([Hd, L], bf16, name="sin_sb")

            # ==== phase 1: fused q/k/v projections + RoPE ====================
            with (
                tc.tile_pool(name="wqkv", bufs=1) as wqkv,
                tc.tile_pool(name="stream", bufs=2) as stream,
                tc.tile_pool(name="tmp", bufs=2) as tmp,
                tc.tile_pool(name="pp", bufs=1, space="PSUM") as pp,
            ):
                # first htk block ahead of everything else on the sync queue;
                # leading chunks are fine-grained so the first matmul can
                # start as soon as ktile 0 and wq ktile 0 land
                htk0 = stream.tile([128, KT, 512], bf16, name="htk")
                for kg in range(4):
                    nc.sync.dma_start(
                        out=htk0[:, kg * 4 : (kg + 1) * 4, :],
                        in_=hTt[0, :, kg * 4 : (kg + 1) * 4, :],
                    )
                # weight prefetch, tag-major: q is consumed first (+4us),
                # then k (+17us), then v (+31us)
                w_sbs = {}
                for tag in ("q", "k", "v"):
                    w_sbs[tag] = wqkv.tile([128, KT, QKV], bf16, name=f"w_{tag}")
                w_res = {"q": wqT, "k": wkT, "v": wvT}
                for tag in ("q", "k", "v"):
                    w_re = w_res[tag]
                    for a, b in (
                        (0, 1), (1, 2), (2, 4), (4, 6), (6, 8),
                        (8, 10), (10, 12), (12, 16),
                    ):
                        nc.gpsimd.dma_start(
                            out=w_sbs[tag][:, a:b, :], in_=w_re[:, a:b, :]
                        )
                # rope constants: needed at the first flush (~14us in)
                nc.sync.dma_start(out=cos_sb, in_=cosT)
                nc.sync.dma_start(out=sin_sb, in_=sinTs)

                # PE warm-up: junk matmuls during the initial DMA wait so the
                # clock ramp (free-running ~3.4us activity window) completes
                # before real data lands
                ps_warm = pp.tile([128, 512], f32, name="pp0", bufs=2)
                for _ in range(7):
                    nc.tensor.matmul(
                        ps_warm, warm[:, 0:128], warm, start=True, stop=True
                    )

                # deferred rope work: list of closures, emitted one per kg
                # boundary of the *following* projection stream so the ACT
                # queue (which also drains the PSUM flush copies) never
                # backs up.
                pending_rope = []

                def emit_one_rope():
                    if pending_rope:
                        pending_rope.pop(0)()

                def make_rope(dst, m, n, raw):
                    csl = slice(n * 512, (n + 1) * 512)

                    def do():
                        # rotate-half permutation: two partition-offset
                        # copies on ACT (sign already folded into sinTs)
                        rr = tmp.tile([128, 512], bf16, name="rr")
                        nc.scalar.copy(rr[0:64, :], raw[64:128, :])
                        nc.scalar.copy(rr[64:128, :], raw[0:64, :])
                        t1 = tmp.tile([128, 512], bf16, name="t1")
                        nc.vector.tensor_mul(t1, raw, cos_sb[:, csl])
                        t2 = tmp.tile([128, 512], bf16, name="t2")
                        nc.vector.tensor_mul(t2, rr, sin_sb[:, csl])
                        nc.gpsimd.tensor_add(dst[m][:, csl], t1, t2)

                    return do

                for n in range(NT):
                    if n == 0:
                        htk = htk0
                    else:
                        htk = stream.tile([128, KT, 512], bf16, name="htk")
                        for kg in range(2):
                            nc.sync.dma_start(
                                out=htk[:, kg * 8 : (kg + 1) * 8, :],
                                in_=hTt[n, :, kg * 8 : (kg + 1) * 8, :],
                            )
                    for tag, dst in (("q", qT), ("k", kT), ("v", None)):
                        w_sb = w_sbs[tag]
                        ps_x = [
                            pp.tile(
                                [128, 512], f32, name=f"pp{m}",
                                bufs=2 if m < 2 else 1,
                            )
                            for m in range(TPH)
                        ]
                        if tag == "v":
                            # v: stationary = token columns of htk, moving = wv
                            # rows; out [token128, qkv512] per token sub-tile.
                            for kg in range(4):
                                for mc in range(4):
                                    for i in range(4):
                                        kk = kg * 4 + i
                                        nc.tensor.matmul(
                                            ps_x[mc],
                                            htk[:, kk, mc * 128 : (mc + 1) * 128],
                                            w_sb[:, kk, :],
                                            start=(kk == 0),
                                            stop=(kk == KT - 1),
                                        )
                                emit_one_rope()
                            for mc in range(4):
                                if mc < 2:
                                    nc.scalar.copy(v_big[:, n * 4 + mc, :], ps_x[mc])
                                else:
                                    nc.vector.tensor_copy(
                                        v_big[:, n * 4 + mc, :], ps_x[mc]
                                    )
                        else:
                            # q/k: stationary = weight m-tile, moving = htk.
                            # m-major within each kg so the first matmul of
                            # m2/m3 (single-buffered banks) comes late enough
                            # for the previous flush to have freed them.
                            for kg in range(4):
                                for m in range(TPH):
                                    for i in range(4):
                                        kk = kg * 4 + i
                                        nc.tensor.matmul(
                                            ps_x[m],
                                            w_sb[:, kk, m * 128 : (m + 1) * 128],
                                            htk[:, kk, :],
                                            start=(kk == 0),
                                            stop=(kk == KT - 1),
                                        )
                                emit_one_rope()
                            # flush: raw copies split ACT/DVE, rope deferred
                            raws = []
                            for m in range(TPH):
                                raw = tmp.tile([128, 512], bf16, name="raw", bufs=4)
                                if m < 2:
                                    nc.scalar.copy(raw, ps_x[m])
                                else:
                                    nc.vector.tensor_copy(raw, ps_x[m])
                                raws.append(raw)
                            for m in range(TPH):
                                pending_rope.append(make_rope(dst, m, n, raws[m]))
                while pending_rope:
                    emit_one_rope()

            # ==== phase 2+3: attention with fused o_proj =====================
            with (
                tc.tile_pool(name="wo", bufs=1) as wop,
                tc.tile_pool(name="att", bufs=2) as att,
            ):
                wo_sb = wop.tile([128, TPH, D], bf16, name="wo_sb")
                for hh in range(TPH):
                    nc.gpsimd.dma_start(out=wo_sb[:, hh, :], in_=woT[:, hh, :])

                HW = 1024  # tq half-width

                seq = [
                    (half, h, tk)
                    for half in range(2)
                    for h in range(TPH)
                    for tk in range(MT)
                ]
                n_seq = len(seq)
                state = {}  # (half,h) -> ps_out
                probs_by_idx = {}
                partials = {}
                # partial index by tk: p0 = tk0-7, p1 = tk8-15 (all on DVE —
                # a GpSimd hop here once put a multi-us library-op latency
                # in front of the den matmul and stalled the in-order PE
                # queue at every unit boundary)
                P_OF_TK = [0] * 8 + [1] * 8

                PIPE = 2  # av trails scores by 2 steps so exp() is done
                with (
                    tc.tile_pool(name="pss", bufs=2, space="PSUM") as pss,
                    tc.tile_pool(name="pso", bufs=2, space="PSUM") as pso,
                ):
                    def front(t):
                        half, h, tk = seq[t]
                        # full-width scores tile (2 banks); each 512-wide
                        # matmul is single-shot (start&stop) into its own
                        # bank. One 1024-wide exp serves both. wq is
                        # pre-scaled by 1/sqrt(Hd) on the host.
                        sc_ps = pss.tile([128, HW], f32, name="sc")
                        for j in range(2):
                            tq0 = half * HW + j * 512
                            nc.tensor.matmul(
                                sc_ps[:, j * 512 : (j + 1) * 512],
                                kT[h][:, tk * 128 : (tk + 1) * 128],
                                qT[h][:, tq0 : tq0 + 512],
                                start=True,
                                stop=True,
                            )
                        probs = att.tile([128, HW], bf16, name="probs", bufs=5)
                        probs_by_idx[t] = probs
                        nc.scalar.activation(probs, sc_ps, AF.Exp)

                    def back_av(t):
                        half, h, tk = seq[t]
                        if (half, h) not in state:
                            state[(half, h)] = pso.tile([Hd, HW], f32, name="ps_out")
                        ps_out = state[(half, h)]
                        probs = probs_by_idx.pop(t)
                        st = dict(start=(tk == 0), stop=(tk == MT - 1))
                        for j in range(2):
                            nc.tensor.matmul(
                                ps_out[:, j * 512 : (j + 1) * 512],
                                v_big[:, tk, h * 128 : (h + 1) * 128],
                                probs[:, j * 512 : (j + 1) * 512],
                                **st,
                            )
                        # denominator: accumulate probs tiles on the DVE into
                        # 3 bf16 partial sums; GpSimd pre-reduces those to
                        # one, so the PE only does 2 single-shot ones-matmuls
                        # per unit. bf16 partial rounding adds ~0.2% den
                        # error.
                        p_idx = P_OF_TK[tk]
                        parts = partials.setdefault((half, h), [None] * 2)
                        if parts[p_idx] is None:
                            pt = att.tile(
                                [128, HW], bf16, name=f"part{p_idx}", bufs=2
                            )
                            parts[p_idx] = pt
                            nc.vector.tensor_copy(pt, probs)
                        else:
                            pt = parts[p_idx]
                            nc.vector.tensor_add(pt, pt, probs)

                    def tail_a(t):
                        half, h, tk = seq[t]
                        ps_out = state[(half, h)]
                        # finish the partial combine, then copy out the
                        # unnormalized attention output; both on DVE so the
                        # den matmul's dependencies resolve without any
                        # cross-engine latency
                        parts = partials[(half, h)]
                        nc.vector.tensor_add(parts[0], parts[0], parts[1])
                        sl = slice(half * HW, (half + 1) * HW)
                        nc.vector.tensor_copy(outT[h][:, sl], ps_out)

                    def tail_b(t):
                        half, h, tk = seq[t]
                        ps_out = state.pop((half, h))
                        parts = partials.pop((half, h))
                        # denominator row: 2 single-shot ones-matmuls into
                        # ps_out[0:1] (bank already copied out by tail_a, so
                        # no dedicated PSUM banks for the denominator)
                        den_ap = ps_out[0:1, :]
                        for j in range(2):
                            nc.tensor.matmul(
                                den_ap[:, j * 512 : (j + 1) * 512],
                                ones_b,
                                parts[0][:, j * 512 : (j + 1) * 512],
                                start=True,
                                stop=True,
                            )
                        # fast reciprocal on DVE, partition broadcast on
                        # GpSimd, in-place DVE normalize
                        sl = slice(half * HW, (half + 1) * HW)
                        rec = att.tile([1, HW], f32, name="rec", bufs=2)
                        nc.vector.reciprocal_approx_fast(rec, den_ap)
                        rec_bc = att.tile([128, HW], f32, name="rec_bc", bufs=2)
                        nc.gpsimd.partition_broadcast(rec_bc, rec, channels=128)
                        nc.vector.tensor_mul(
                            outT[h][:, sl], outT[h][:, sl], rec_bc
                        )

                    # ---- o_proj work quanta -------------------------------
                    # one quantum = one (m, nblk-pair): 2 accumulation chains
                    # of 4 matmuls each, in the 2 banks of a PSUM slot
                    # borrowed from the attention pools (rotating over all 4
                    # slot-groups = 8 banks, so chains never wait on the
                    # previous quantum's copies), + 2 PSUM->SBUF copies; DMA
                    # per m.
                    ot4_by_m = {}
                    oproj_copy_n = [0]

                    def oproj_quantum(m, pair, engines, pool, tname):
                        of = pool.tile([128, HW], f32, name=tname)
                        if pair == 0:
                            ot4_by_m[m] = att.tile(
                                [128, 4, 512], bf16, name="ot4", bufs=3
                            )
                        ot4 = ot4_by_m[m]
                        for nblk in (2 * pair, 2 * pair + 1):
                            bsl = slice((nblk % 2) * 512, (nblk % 2) * 512 + 512)
                            for hh in range(TPH):
                                nc.tensor.matmul(
                                    of[:, bsl],
                                    outT[hh][:, m * 128 : (m + 1) * 128],
                                    wo_sb[:, hh, nblk * 512 : (nblk + 1) * 512],
                                    start=(hh == 0),
                                    stop=(hh == TPH - 1),
                                )
                        for nblk in (2 * pair, 2 * pair + 1):
                            bsl = slice((nblk % 2) * 512, (nblk % 2) * 512 + 512)
                            eng = engines[oproj_copy_n[0] % len(engines)]
                            oproj_copy_n[0] += 1
                            if eng == "v":
                                nc.vector.tensor_copy(ot4[:, nblk, :], of[:, bsl])
                            elif eng == "s":
                                nc.scalar.copy(ot4[:, nblk, :], of[:, bsl])
                            else:
                                nc.gpsimd.tensor_copy(ot4[:, nblk, :], of[:, bsl])
                        if pair == 1:
                            ot4 = ot4_by_m.pop(m)
                            if m == MT - 1:
                                # last tile: per-d-block DMAs to shorten the
                                # final drain
                                for nblk in range(4):
                                    nc.sync.dma_start(
                                        out=out_re[:, m, nblk, :],
                                        in_=ot4[:, nblk, :],
                                    )
                            else:
                                nc.sync.dma_start(out=out_re[:, m, :, :], in_=ot4)

                    # 2-wide supersteps: sc pairs for (s, s+1) then av pairs
                    # for (s-2, s-1). A unit's tail is split: tail_a (DVE
                    # copy-out + final partial combine) runs with its
                    # superstep; tail_b (den matmuls + recip + broadcast +
                    # normalize) is delayed one superstep so the den matmul
                    # reaches the front of the in-order PE queue only after
                    # its DVE dependencies have resolved. ps_out is
                    # double-buffered, so the next unit accumulates into the
                    # other slot while the tail chain drains.
                    pending_tail_b = []  # (due_superstep, t)
                    for s in range(0, n_seq + 8, 2):
                        for t in (s, s + 1):
                            if t < n_seq:
                                front(t)
                        for t in (s - PIPE, s - PIPE + 1):
                            if 0 <= t < n_seq:
                                back_av(t)
                        while pending_tail_b and pending_tail_b[0][0] <= s:
                            tail_b(pending_tail_b.pop(0)[1])
                        for t in (s - PIPE, s - PIPE + 1):
                            if 0 <= t < n_seq and seq[t][2] == MT - 1:
                                tail_a(t)
                                pending_tail_b.append((s + 6, t))
                    # o_proj: rotate quanta over the four freed PSUM slot
                    # groups; copies alternate ACT/DVE now that the exp
                    # stream is done
                    slot_cycle = [(pss, "sc"), (pss, "sc"), (pso, "ps_out"), (pso, "ps_out")]
                    qn = 0
                    for m in range(MT):
                        for pair in range(2):
                            pool, tname = slot_cycle[qn % 4]
                            qn += 1
                            oproj_quantum(m, pair, ("s", "v"), pool, tname)

    nc.compile()
    return nc


def _bf(x: np.ndarray) -> np.ndarray:
    return np.ascontiguousarray(x, dtype=np.float32).astype(ml_dtypes.bfloat16)


def kernel(hidden_states, cos, sin, wq, wk, wv, wo):
    if "nc" not in _CACHE:
        _CACHE["nc"] = _build()
    nc = _CACHE["nc"]

    hidden_states = np.asarray(hidden_states, dtype=np.float32)
    cos = np.asarray(cos, dtype=np.float32)
    sin = np.asarray(sin, dtype=np.float32)
    wq = np.asarray(wq, dtype=np.float32)
    wk = np.asarray(wk, dtype=np.float32)
    wv = np.asarray(wv, dtype=np.float32)
    wo = np.asarray(wo, dtype=np.float32)

    # host-side layout prep
    cosT = _bf(cos[0, 0].T)                             # [Hd, L]
    sinT = np.ascontiguousarray(sin[0, 0].T)            # [Hd, L]
    sinTs = sinT.copy()
    sinTs[: Hd // 2] *= -1.0                            # fold rotate_half signs
    sinTs = _bf(sinTs)

    # pre-tile for contiguous per-partition DMA lines:
    #   hTt[n, p, kk, t] = h.T[kk*128+p, n*512+t]
    #   w*T[p, kk, r]    = w[r, kk*128+p]   (w.T row d = kk*128+p)
    #   woT[p, hh, d]    = wo[d, r0+hh*128+p]
    def _tile_h(hb):
        return _bf(
            hb.T.reshape(KT, 128, NT, 512).transpose(2, 1, 0, 3)
        )

    def _tile_w(wrows):
        return _bf(wrows.T.reshape(KT, 128, QKV).transpose(1, 0, 2))

    def _tile_wo(wcols):
        return _bf(wcols.T.reshape(TPH, 128, D).transpose(1, 0, 2))

    hTt = [_tile_h(hidden_states[b]) for b in range(B)]
    # fold the attention scale into wq so raw scores are ~N(0,1) (keeps the
    # bf16 PSUM score rounding relative) and exp() needs no scale
    wq = wq * SCALE

    in_maps = []
    for c in range(NC):
        b = c // 4
        hb = c % 4
        r0 = hb * QKV
        in_maps.append(
            {
                "hTt": hTt[b],
                "wqT": _tile_w(wq[r0 : r0 + QKV]),
                "wkT": _tile_w(wk[r0 : r0 + QKV]),
                "wvT": _tile_w(wv[r0 : r0 + QKV]),
                "woT": _tile_wo(wo[:, r0 : r0 + QKV]),
                "cosT": cosT,
                "sinTs": sinTs,
            }
        )

    res = run_bass_kernel_spmd(nc, in_maps, core_ids=list(range(NC)))
    _CACHE["last_results"] = res

    out = np.zeros((B, L, D), dtype=np.float32)
    for c in range(NC):
        out[c // 4] += np.asarray(res.results[c]["out"], dtype=np.float32)
    return out


# revision 17
# speedup vs baseline: 1.2989x; 1.0089x over previous
"""Multi-head attention layer (QKV proj + RoPE + SDPA + o_proj) on 8 TRN2 cores.

Sharding: DP2 x TP4. Core c handles batch c//4 and heads 4*(c%4)..4*(c%4)+4.
Each core computes its 4 heads' attention and a partial o_proj output
[L, D]; the host sums the 4 partials per batch (row-parallel o_proj).

All matmul operands are bf16 (same 1 cycle/row PE rate as fp32r on TRN2,
half the DMA/SBUF footprint); PSUM accumulation is fp32 throughout.

Structure (single PE stream, minimal gaps). Phase 1 runs TAG-MAJOR (all
q projections, then all k, then all v) so that by the time the v
projections stream, qT/kT are fully roped and the first attention
score+exp supersteps can interleave into the v-pass — the ACT exp
stream (the binding engine of the attention phase, ~1.1us per
[128,1024] exp) gets an 8-step head start on otherwise-idle ACT time.
hidden_states is re-streamed from DRAM per pass (3x8MB, far under DMA
capacity); wq/wk/wv share two rotating SBUF slots so the v weights
land during the k-pass without reserving a third 16KB/partition slot.

  q/k pass: for each 512-token block n, stream htk and run 16 matmuls
           per kg into 4 PSUM accumulation chains (m-major within kg so
           single-buffered banks flush in time). RoPE is PE-free: the
           rotate-half permutation is two ACT partition-offset copies
           (sign folded into sin host-side), cos/sin multiplies on DVE
           in bf16 (2x mode), final add on GpSimd. Rope units are
           deferred one kg boundary so no engine queue piles up.
  v pass:  one 16-matmul chain per 128-token column, ping-ponging 2
           PSUM banks; after every other chain one attention front
           (2 score matmuls + 1 exp) is interleaved.
  attention: 2-kv-tile supersteps; av trails scores by 8 steps (the
           v-pass head start), probs buffered 10 deep. Scores land in a
           [128,1024] fp32 PSUM tile (two single-shot 512-wide matmuls
           -> one 1024-wide exp; wq carries the 1/sqrt(Hd) scale).
           Softmax skips max-subtraction (scores ~N(0,1)). Denominator:
           probs accumulate into 2 bf16 partials on DVE (combined on
           DVE at unit end — never a GpSimd hop, whose multi-us op
           latency would gate the in-order PE queue); the PE only does
           2 single-shot ones-matmuls per (head, half), written into
           ps_out[0:1] after its copy-out, emitted 3 supersteps late so
           the in-order PE queue never waits on them. ps_out is
           double-buffered so the next unit accumulates while the tail
           (DVE reciprocal -> gpsimd.partition_broadcast -> DVE
           normalize) drains.
  o_proj:  each (token-tile, d-block-pair) quantum runs its two
           4-matmul accumulation chains in a PSUM slot borrowed
           round-robin from the attention pools (8 banks in rotation),
           starting while the last attention tails drain. bf16 output
           DMA (summed to fp32 on the host); the last token tile DMAs
           per-d-block to shorten the drain.

Accumulation-chain rule learned the hard way: `start=True` clears the
has_written bits for the WHOLE PSUM bank, so two interleaved multi-step
accumulation chains must never share a bank (single-shot matmuls may).
"""

import numpy as np

import sys
import types

# Defensive: concourse.bass_utils imports antenv.axon_hooks when tracing is
# requested; provide a null shim if the module is absent in this image so a
# stray BASS_TRACE env var cannot crash the kernel.
try:
    import antenv.axon_hooks  # noqa: F401
except ImportError:
    _m = types.ModuleType("antenv.axon_hooks")
    _m.set_axon_ntff_profile_hook = lambda h: None
    _m.get_axon_ntff_profile_hook = lambda: None
    sys.modules["antenv.axon_hooks"] = _m

import ml_dtypes

import concourse.bass as bass
import concourse.mybir as mybir
import concourse.tile as tile
from concourse import bacc
from concourse.bass_utils import run_bass_kernel_spmd

# problem constants (hardcoded per spec)
B, L, D = 2, 2048, 2048
H, Hd = 16, 128
NC = 8
TPH = 4            # heads per core
QKV = TPH * Hd     # 512 per-core projection width
KT = D // 128      # 16 contraction tiles
NT = L // 512      # 4 token groups of 512
MT = L // 128      # 16 token chunks of 128

f32 = mybir.dt.float32
bf16 = mybir.dt.bfloat16

AF = mybir.ActivationFunctionType
SCALE = 1.0 / float(np.sqrt(Hd))

_CACHE: dict = {}


def _build():
    nc = bacc.Bacc("TRN2", target_bir_lowering=False, debug=False)

    # inputs are pre-tiled on the host so every DMA line is contiguous per
    # partition (4-16KB instead of 1KB)
    hTt = nc.dram_tensor("hTt", [NT, 128, KT, 512], bf16, kind="ExternalInput").ap()
    wqT = nc.dram_tensor("wqT", [128, KT, QKV], bf16, kind="ExternalInput").ap()
    wkT = nc.dram_tensor("wkT", [128, KT, QKV], bf16, kind="ExternalInput").ap()
    wvT = nc.dram_tensor("wvT", [128, KT, QKV], bf16, kind="ExternalInput").ap()
    woT = nc.dram_tensor("woT", [128, TPH, D], bf16, kind="ExternalInput").ap()
    cosT = nc.dram_tensor("cosT", [Hd, L], bf16, kind="ExternalInput").ap()
    sinTs = nc.dram_tensor("sinTs", [Hd, L], bf16, kind="ExternalInput").ap()
    out = nc.dram_tensor("out", [L, D], bf16, kind="ExternalOutput").ap()

    out_re = out.rearrange("(mm p) (nb d) -> p mm nb d", p=128, d=512)

    HW = 1024   # tq half-width
    F = 8       # attention fronts interleaved into the v-pass (= av lag)

    with tile.TileContext(nc) as tc:
        with tc.tile_pool(name="persist", bufs=1) as persist:
            # ---- persistent tensors -----------------------------------
            ones_b = persist.tile([128, 1], bf16, name="ones_b")
            nc.vector.memset(ones_b, 1.0)
            warm = persist.tile([128, 512], bf16, name="warm")
            nc.vector.memset(warm, 0.0)
            qT = [persist.tile([Hd, L], bf16, name=f"qT{h}") for h in range(TPH)]
            kT = [persist.tile([Hd, L], bf16, name=f"kT{h}") for h in range(TPH)]
            v_big = persist.tile([128, MT, QKV], bf16, name="v_big")
            outT = [persist.tile([Hd, L], bf16, name=f"outT{h}") for h in range(TPH)]
            cos_sb = persist.tile([Hd, L], bf16, name="cos_sb")
            sin_sb = persist.tile([Hd, L], bf16, name="sin_sb")

            with (
                tc.tile_pool(name="wsh", bufs=2) as wsh,
                tc.tile_pool(name="stream", bufs=2) as stream,
                tc.tile_pool(name="tmp", bufs=2) as tmp,
                tc.tile_pool(name="wo", bufs=1) as wop,
                tc.tile_pool(name="att", bufs=2) as att,
            ):
                # ---- prefetch wave: first htk block + q/k weights ------
                htk0 = stream.tile([128, KT, 512], bf16, name="htk")
                for kg in range(4):
                    nc.sync.dma_start(
                        out=htk0[:, kg * 4 : (kg + 1) * 4, :],
                        in_=hTt[0, :, kg * 4 : (kg + 1) * 4, :],
                    )
                # w slots rotate: q -> slot A, k -> slot B, v -> slot A
                # again (the v DMA naturally waits for the q-pass readers)
                w_q = wsh.tile([128, KT, QKV], bf16, name="w")
                for a, b in (
                    (0, 1), (1, 2), (2, 4), (4, 6), (6, 8),
                    (8, 10), (10, 12), (12, 16),
                ):
                    nc.gpsimd.dma_start(out=w_q[:, a:b, :], in_=wqT[:, a:b, :])
                w_k = wsh.tile([128, KT, QKV], bf16, name="w")
                for a, b in ((0, 4), (4, 8), (8, 12), (12, 16)):
                    nc.gpsimd.dma_start(out=w_k[:, a:b, :], in_=wkT[:, a:b, :])
                # rope constants: first half needed at the first flush
                nc.sync.dma_start(out=cos_sb[:, 0:HW], in_=cosT[:, 0:HW])
                nc.sync.dma_start(out=sin_sb[:, 0:HW], in_=sinTs[:, 0:HW])

                # deferred rope work: list of closures, emitted one per kg
                # boundary of the *following* projection stream so the ACT
                # queue (which also drains the PSUM flush copies) never
                # backs up.
                pending_rope = []

                def emit_one_rope():
                    if pending_rope:
                        pending_rope.pop(0)()

                def make_rope(dst, m, n, raw):
                    csl = slice(n * 512, (n + 1) * 512)

                    def do():
                        # rotate-half permutation: two partition-offset
                        # copies on ACT (sign already folded into sinTs)
                        rr = tmp.tile([128, 512], bf16, name="rr")
                        nc.scalar.copy(rr[0:64, :], raw[64:128, :])
                        nc.scalar.copy(rr[64:128, :], raw[0:64, :])
                        t1 = tmp.tile([128, 512], bf16, name="t1")
                        nc.vector.tensor_mul(t1, raw, cos_sb[:, csl])
                        t2 = tmp.tile([128, 512], bf16, name="t2")
                        nc.vector.tensor_mul(t2, rr, sin_sb[:, csl])
                        nc.gpsimd.tensor_add(dst[m][:, csl], t1, t2)

                    return do

                # ==== q / k passes ======================================
                with tc.tile_pool(name="pp", bufs=1, space="PSUM") as pp:
                    # PE warm-up: junk matmuls during the initial DMA wait
                    # so the clock ramp completes before real data lands
                    ps_warm = pp.tile([128, 512], f32, name="pp0", bufs=2)
                    for _ in range(7):
                        nc.tensor.matmul(
                            ps_warm, warm[:, 0:128], warm, start=True, stop=True
                        )

                    def qk_pass(w_sb, dst, first):
                        for n in range(NT):
                            if first and n == 0:
                                htk = htk0
                            else:
                                htk = stream.tile([128, KT, 512], bf16, name="htk")
                                for kg in range(2):
                                    nc.sync.dma_start(
                                        out=htk[:, kg * 8 : (kg + 1) * 8, :],
                                        in_=hTt[n, :, kg * 8 : (kg + 1) * 8, :],
                                    )
                            if first and n == 1:
                                # second half of the rope constants
                                nc.sync.dma_start(
                                    out=cos_sb[:, HW:L], in_=cosT[:, HW:L]
                                )
                                nc.sync.dma_start(
                                    out=sin_sb[:, HW:L], in_=sinTs[:, HW:L]
                                )
                            ps_x = [
                                pp.tile(
                                    [128, 512], f32, name=f"pp{m}",
                                    bufs=2 if m < 2 else 1,
                                )
                                for m in range(TPH)
                            ]
                            # stationary = weight m-tile, moving = htk.
                            # m-major within each kg so the first matmul of
                            # m2/m3 (single-buffered banks) comes late
                            # enough for the previous flush to have freed
                            # them.
                            for kg in range(4):
                                for m in range(TPH):
                                    for i in range(4):
                                        kk = kg * 4 + i
                                        nc.tensor.matmul(
                                            ps_x[m],
                                            w_sb[:, kk, m * 128 : (m + 1) * 128],
                                            htk[:, kk, :],
                                            start=(kk == 0),
                                            stop=(kk == KT - 1),
                                        )
                                emit_one_rope()
                            # flush: raw copies split ACT/DVE, rope deferred
                            raws = []
                            for m in range(TPH):
                                raw = tmp.tile([128, 512], bf16, name="raw", bufs=4)
                                if m < 2:
                                    nc.scalar.copy(raw, ps_x[m])
                                else:
                                    nc.vector.tensor_copy(raw, ps_x[m])
                                raws.append(raw)
                            for m in range(TPH):
                                pending_rope.append(make_rope(dst, m, n, raws[m]))

                    qk_pass(w_q, qT, True)
                    # v weights into slot A (waits q-pass readers via the
                    # tile framework; by the time the gpsimd queue reaches
                    # these DGE ops the q-pass is done, so nothing blocks),
                    # plus o_proj weights on the sync queue
                    w_v = wsh.tile([128, KT, QKV], bf16, name="w")
                    for a, b in ((0, 4), (4, 8), (8, 12), (12, 16)):
                        nc.gpsimd.dma_start(out=w_v[:, a:b, :], in_=wvT[:, a:b, :])
                    qk_pass(w_k, kT, False)

                # ==== attention machinery ===============================
                wo_sb = wop.tile([128, TPH, D], bf16, name="wo_sb")

                seq = [
                    (half, h, tk)
                    for half in range(2)
                    for h in range(TPH)
                    for tk in range(MT)
                ]
                n_seq = len(seq)
                state = {}  # (half,h) -> ps_out
                probs_by_idx = {}
                partials = {}
                # partial index by tk: p0 = tk0-7, p1 = tk8-15 (all on DVE)
                P_OF_TK = [0] * 8 + [1] * 8

                with tc.tile_pool(name="pss", bufs=2, space="PSUM") as pss:
                    def front(t):
                        half, h, tk = seq[t]
                        # full-width scores tile (2 banks); each 512-wide
                        # matmul is single-shot (start&stop) into its own
                        # bank. One 1024-wide exp serves both. wq is
                        # pre-scaled by 1/sqrt(Hd) on the host.
                        sc_ps = pss.tile([128, HW], f32, name="sc")
                        for j in range(2):
                            tq0 = half * HW + j * 512
                            nc.tensor.matmul(
                                sc_ps[:, j * 512 : (j + 1) * 512],
                                kT[h][:, tk * 128 : (tk + 1) * 128],
                                qT[h][:, tq0 : tq0 + 512],
                                start=True,
                                stop=True,
                            )
                        probs = att.tile([128, HW], bf16, name="probs", bufs=F + 2)
                        probs_by_idx[t] = probs
                        nc.scalar.activation(probs, sc_ps, AF.Exp)

                    # ==== v pass, with the first F fronts interleaved ====
                    with tc.tile_pool(name="pv", bufs=2, space="PSUM") as pv:
                        next_front = [0]
                        for n in range(NT):
                            htk = stream.tile([128, KT, 512], bf16, name="htk")
                            for kg in range(2):
                                nc.sync.dma_start(
                                    out=htk[:, kg * 8 : (kg + 1) * 8, :],
                                    in_=hTt[n, :, kg * 8 : (kg + 1) * 8, :],
                                )
                            if n == 0:
                                for hh in range(TPH):
                                    nc.sync.dma_start(
                                        out=wo_sb[:, hh, :], in_=woT[:, hh, :]
                                    )
                            for mc in range(4):
                                ps_v = pv.tile([128, 512], f32, name="pv")
                                for kk in range(KT):
                                    nc.tensor.matmul(
                                        ps_v,
                                        htk[:, kk, mc * 128 : (mc + 1) * 128],
                                        w_v[:, kk, :],
                                        start=(kk == 0),
                                        stop=(kk == KT - 1),
                                    )
                                if (n * 4 + mc) % 2 == 0:
                                    nc.scalar.copy(v_big[:, n * 4 + mc, :], ps_v)
                                else:
                                    nc.vector.tensor_copy(
                                        v_big[:, n * 4 + mc, :], ps_v
                                    )
                                emit_one_rope()
                                if (n * 4 + mc) % 2 == 1 and next_front[0] < F:
                                    front(next_front[0])
                                    next_front[0] += 1
                        while pending_rope:
                            emit_one_rope()

                    # ==== main attention loop + o_proj ===================
                    with tc.tile_pool(name="pso", bufs=2, space="PSUM") as pso:
                        def back_av(t):
                            half, h, tk = seq[t]
                            if (half, h) not in state:
                                state[(half, h)] = pso.tile(
                                    [Hd, HW], f32, name="ps_out"
                                )
                            ps_out = state[(half, h)]
                            probs = probs_by_idx.pop(t)
                            st = dict(start=(tk == 0), stop=(tk == MT - 1))
                            for j in range(2):
                                nc.tensor.matmul(
                                    ps_out[:, j * 512 : (j + 1) * 512],
                                    v_big[:, tk, h * 128 : (h + 1) * 128],
                                    probs[:, j * 512 : (j + 1) * 512],
                                    **st,
                                )
                            p_idx = P_OF_TK[tk]
                            parts = partials.setdefault((half, h), [None] * 2)
                            if parts[p_idx] is None:
                                pt = att.tile(
                                    [128, HW], bf16, name=f"part{p_idx}", bufs=2
                                )
                                parts[p_idx] = pt
                                nc.vector.tensor_copy(pt, probs)
                            else:
                                pt = parts[p_idx]
                                nc.vector.tensor_add(pt, pt, probs)

                        def tail_a(t):
                            half, h, tk = seq[t]
                            ps_out = state[(half, h)]
                            # finish the partial combine, then copy out the
                            # unnormalized attention output; both on DVE so
                            # the den matmul's dependencies resolve without
                            # any cross-engine latency
                            parts = partials[(half, h)]
                            nc.vector.tensor_add(parts[0], parts[0], parts[1])
                            sl = slice(half * HW, (half + 1) * HW)
                            nc.vector.tensor_copy(outT[h][:, sl], ps_out)

                        def tail_b(t):
                            half, h, tk = seq[t]
                            ps_out = state.pop((half, h))
                            parts = partials.pop((half, h))
                            # denominator row: 2 single-shot ones-matmuls
                            # into ps_out[0:1] (bank already copied out by
                            # tail_a, so no dedicated PSUM banks for the
                            # denominator)
                            den_ap = ps_out[0:1, :]
                            for j in range(2):
                                nc.tensor.matmul(
                                    den_ap[:, j * 512 : (j + 1) * 512],
                                    ones_b,
                                    parts[0][:, j * 512 : (j + 1) * 512],
                                    start=True,
                                    stop=True,
                                )
                            # fast reciprocal on DVE, partition broadcast on
                            # GpSimd, in-place DVE normalize
                            sl = slice(half * HW, (half + 1) * HW)
                            rec = att.tile([1, HW], f32, name="rec", bufs=1)
                            nc.vector.reciprocal_approx_fast(rec, den_ap)
                            rec_bc = att.tile([128, HW], f32, name="rec_bc", bufs=1)
                            nc.gpsimd.partition_broadcast(rec_bc, rec, channels=128)
                            nc.vector.tensor_mul(
                                outT[h][:, sl], outT[h][:, sl], rec_bc
                            )

                        # ---- o_proj work quanta ---------------------------
                        ot4_by_m = {}
                        oproj_copy_n = [0]

                        def oproj_quantum(m, pair, engines, pool, tname):
                            of = pool.tile([128, HW], f32, name=tname)
                            if pair == 0:
                                ot4_by_m[m] = att.tile(
                                    [128, 4, 512], bf16, name="ot4", bufs=2
                                )
                            ot4 = ot4_by_m[m]
                            for nblk in (2 * pair, 2 * pair + 1):
                                bsl = slice(
                                    (nblk % 2) * 512, (nblk % 2) * 512 + 512
                                )
                                for hh in range(TPH):
                                    nc.tensor.matmul(
                                        of[:, bsl],
                                        outT[hh][:, m * 128 : (m + 1) * 128],
                                        wo_sb[:, hh, nblk * 512 : (nblk + 1) * 512],
                                        start=(hh == 0),
                                        stop=(hh == TPH - 1),
                                    )
                            for nblk in (2 * pair, 2 * pair + 1):
                                bsl = slice(
                                    (nblk % 2) * 512, (nblk % 2) * 512 + 512
                                )
                                eng = engines[oproj_copy_n[0] % len(engines)]
                                oproj_copy_n[0] += 1
                                if eng == "v":
                                    nc.vector.tensor_copy(
                                        ot4[:, nblk, :], of[:, bsl]
                                    )
                                else:
                                    nc.scalar.copy(ot4[:, nblk, :], of[:, bsl])
                            if pair == 1:
                                ot4 = ot4_by_m.pop(m)
                                if m == MT - 1:
                                    # last tile: per-d-block DMAs to shorten
                                    # the final drain
                                    for nblk in range(4):
                                        nc.sync.dma_start(
                                            out=out_re[:, m, nblk, :],
                                            in_=ot4[:, nblk, :],
                                        )
                                else:
                                    nc.sync.dma_start(
                                        out=out_re[:, m, :, :], in_=ot4
                                    )

                        # supersteps: sc pairs for (s, s+1), av pairs
                        # trailing by F. A unit's tail is split: tail_a
                        # (DVE copy-out + partial combine) runs with its
                        # superstep; tail_b (den matmuls + recip +
                        # broadcast + normalize) is delayed 3 supersteps so
                        # the den matmul reaches the front of the in-order
                        # PE queue only after its DVE dependencies have
                        # resolved.
                        pending_tail_b = []  # (due_superstep, t)
                        for s in range(F, n_seq + F + 8, 2):
                            for t in (s, s + 1):
                                if t < n_seq:
                                    front(t)
                            for t in (s - F, s - F + 1):
                                if 0 <= t < n_seq:
                                    back_av(t)
                            while pending_tail_b and pending_tail_b[0][0] <= s:
                                tail_b(pending_tail_b.pop(0)[1])
                            for t in (s - F, s - F + 1):
                                if 0 <= t < n_seq and seq[t][2] == MT - 1:
                                    tail_a(t)
                                    pending_tail_b.append((s + 6, t))
                        # o_proj: rotate quanta over the four freed PSUM
                        # slot groups; copies alternate ACT/DVE now that
                        # the exp stream is done
                        slot_cycle = [
                            (pss, "sc"), (pss, "sc"),
                            (pso, "ps_out"), (pso, "ps_out"),
                        ]
                        qn = 0
                        for m in range(MT):
                            for pair in range(2):
                                pool, tname = slot_cycle[qn % 4]
                                qn += 1
                                oproj_quantum(m, pair, ("s", "v"), pool, tname)

    nc.compile()
    return nc


def _bf(x: np.ndarray) -> np.ndarray:
    return np.ascontiguousarray(x, dtype=np.float32).astype(ml_dtypes.bfloat16)


def kernel(hidden_states, cos, sin, wq, wk, wv, wo):
    if "nc" not in _CACHE:
        _CACHE["nc"] = _build()
    nc = _CACHE["nc"]

    hidden_states = np.asarray(hidden_states, dtype=np.float32)
    cos = np.asarray(cos, dtype=np.float32)
    sin = np.asarray(sin, dtype=np.float32)
    wq = np.asarray(wq, dtype=np.float32)
    wk = np.asarray(wk, dtype=np.float32)
    wv = np.asarray(wv, dtype=np.float32)
    wo = np.asarray(wo, dtype=np.float32)

    # host-side layout prep
    cosT = _bf(cos[0, 0].T)                             # [Hd, L]
    sinT = np.ascontiguousarray(sin[0, 0].T)            # [Hd, L]
    sinTs = sinT.copy()
    sinTs[: Hd // 2] *= -1.0                            # fold rotate_half signs
    sinTs = _bf(sinTs)

    # pre-tile for contiguous per-partition DMA lines:
    #   hTt[n, p, kk, t] = h.T[kk*128+p, n*512+t]
    #   w*T[p, kk, r]    = w[r, kk*128+p]   (w.T row d = kk*128+p)
    #   woT[p, hh, d]    = wo[d, r0+hh*128+p]
    def _tile_h(hb):
        return _bf(
            hb.T.reshape(KT, 128, NT, 512).transpose(2, 1, 0, 3)
        )

    def _tile_w(wrows):
        return _bf(wrows.T.reshape(KT, 128, QKV).transpose(1, 0, 2))

    def _tile_wo(wcols):
        return _bf(wcols.T.reshape(TPH, 128, D).transpose(1, 0, 2))

    hTt = [_tile_h(hidden_states[b]) for b in range(B)]
    # fold the attention scale into wq so raw scores are ~N(0,1) (keeps the
    # bf16 PSUM score rounding relative) and exp() needs no scale
    wq = wq * SCALE

    in_maps = []
    for c in range(NC):
        b = c // 4
        hb = c % 4
        r0 = hb * QKV
        in_maps.append(
            {
                "hTt": hTt[b],
                "wqT": _tile_w(wq[r0 : r0 + QKV]),
                "wkT": _tile_w(wk[r0 : r0 + QKV]),
                "wvT": _tile_w(wv[r0 : r0 + QKV]),
                "woT": _tile_wo(wo[:, r0 : r0 + QKV]),
                "cosT": cosT,
                "sinTs": sinTs,
            }
        )

    res = run_bass_kernel_spmd(nc, in_maps, core_ids=list(range(NC)))
    _CACHE["last_results"] = res

    out = np.zeros((B, L, D), dtype=np.float32)
    for c in range(NC):
        out[c // 4] += np.asarray(res.results[c]["out"], dtype=np.float32)
    return out
